# revision 1
# baseline (speedup 1.0000x reference)
"""Trainium2 Bass kernel v3: diagonal max-identity pipeline.

Math (per core, one batch):
  c = trig_h + b1  (bf16), nega = -arg_h (bf16, wrap-extended by NT cols)
  m[h, col] = max(c[h,t], nega[h,e])  for col=(r,i): t=i, e=(r+i)%E  [DVE TT 2x]
  out_m = sum_h m*W2 (col-tiled psum accum);  v = sum_h nega*W2
  assist t's (last TA): hid = relu(-nega*-1 + c) per-t on ACT, own psum chunk
  host: out = out_m - v + b2 (m region), out_a + b2 (assist region)
"""

import sys

if "/opt/trn_rl_repo" not in sys.path:
    sys.path.insert(0, "/opt/trn_rl_repo")

import numpy as np
import ml_dtypes

B, T, E, D, H, O = 8, 48, 96, 768, 2048, 2
HT = H // 128
DT2 = (2 * D) // 128
DT = D // 128
NT = 45                   # t's on the DVE diagonal path
TA = T - NT               # t's on the ACT-assist path
MC = E * NT               # diag columns per h-tile
CH = 480                  # mm2 chunk (MC = 9*480)
NCH = MC // CH            # 9 main chunks

_cache = {}

# --- tuning knobs (iterate one at a time) ---
HEAD_DMA = True           # b1 on scalar ring; split xt + first w1 tile
SPLIT_LAST_TT = False     # 3-way split of the last h-tile's TT
NEGA_FIRST = True        # drain order: nega before tb


def _split_excess_waits(nc, mybir, max_waits=1):
    n_split = 0
    for f in nc.m.functions:
        for bb in f.blocks:
            new_insts = []
            for ins in bb.instructions:
                si = getattr(ins, "sync_info", None)
                ow = list(si.on_wait) if (si and si.on_wait) else []
                if len(ow) > max_waits:
                    head, rest = ow[:-max_waits], ow[-max_waits:]
                    for k in range(0, len(head), max_waits):
                        nop = mybir.InstNoOp(
                            name=nc.get_next_instruction_name(), ins=[], outs=[]
                        )
                        nop.engine = ins.engine
                        nop.sync_info = mybir.SyncInfo(
                            on_wait=head[k : k + max_waits], on_update=[]
                        )
                        nop.bass_nofuse = True
                        new_insts.append(nop)
                        n_split += 1
                    si.on_wait = rest
                new_insts.append(ins)
            bb.instructions[:] = new_insts
    return n_split


def _build_nc():
    import concourse.bass as bass
    import concourse.mybir as mybir
    import concourse.tile as tile
    from concourse.bass import AP
    from contextlib import ExitStack

    dt = mybir.dt
    alu = mybir.AluOpType
    act_fn = mybir.ActivationFunctionType

    nc = bass.Bass()
    xt_d = nc.declare_dram_parameter("xt", [128, DT * (T + E)], dt.bfloat16, isOutput=False)
    w1_d = nc.declare_dram_parameter("w1t", [HT, 128, DT2, 128], dt.bfloat16, isOutput=False)
    w2_d = nc.declare_dram_parameter("w2t", [128, HT, O], dt.bfloat16, isOutput=False)
    b1_d = nc.declare_dram_parameter("b1t", [128, HT], dt.float32, isOutput=False)
    outm_d = nc.declare_dram_parameter("outm", [O, MC], dt.float32, isOutput=True)
    outa_d = nc.declare_dram_parameter("outa", [O, TA * E], dt.float32, isOutput=True)
    outv_d = nc.declare_dram_parameter("outv", [O, E], dt.float32, isOutput=True)

    with ExitStack() as ctx:
        tc = ctx.enter_context(tile.TileContext(nc))
        consts = ctx.enter_context(tc.tile_pool(name="consts", bufs=1))
        w1pool = ctx.enter_context(tc.tile_pool(name="w1pool", bufs=4))
        tbpool = ctx.enter_context(tc.tile_pool(name="tbpool", bufs=6))
        npool = ctx.enter_context(tc.tile_pool(name="npool", bufs=6))
        mpool = ctx.enter_context(tc.tile_pool(name="mpool", bufs=4))
        apool = ctx.enter_context(tc.tile_pool(name="apool", bufs=3))
        psA = ctx.enter_context(tc.tile_pool(name="psA", bufs=3, space="PSUM"))
        psB = ctx.enter_context(tc.tile_pool(name="psB", bufs=1, space="PSUM"))

        xt = consts.tile([128, DT * (T + E)], dt.bfloat16)
        w2sb = consts.tile([128, HT, O], dt.bfloat16)
        b1sb = consts.tile([128, HT], dt.float32)

        # Dependency-free dummy Relu: hoists the 1.3us ACT_TABLE_LOAD into
        # the preamble/DMA dead time instead of blocking the first drain.
        warm = consts.tile([128, 2], dt.bfloat16)
        nc.vector.memset(warm[:, 0:1], 0.0)
        nc.scalar.activation(warm[:, 1:2], warm[:, 0:1], act_fn.Relu)

        if HEAD_DMA:
            nc.sync.dma_start(xt[:, 0 : DT * T], xt_d[:, 0 : DT * T])
            w1k0 = w1pool.tile([128, DT2, 128], dt.bfloat16, tag="w1k")
            nc.scalar.dma_start(w1k0[:, 0:DT, :], w1_d[0, :, 0:DT, :])
            nc.sync.dma_start(xt[:, DT * T :], xt_d[:, DT * T :])
            nc.scalar.dma_start(w1k0[:, DT:DT2, :], w1_d[0, :, DT:DT2, :])
            nc.scalar.dma_start(b1sb[:], b1_d[:])
            nc.gpsimd.dma_start(w2sb[:], w2_d[:])
        else:
            nc.sync.dma_start(xt[:], xt_d[:])
            nc.sync.dma_start(w2sb[:], w2_d[:])
            nc.sync.dma_start(b1sb[:], b1_d[:])
            w1k0 = None

        ps2 = [psB.tile([128, CH], dt.float32, name=f"ps2_{i}", tag=f"ps2_{i}") for i in range(3)]

        for k in range(HT):
            if k == 0 and w1k0 is not None:
                w1k = w1k0
            else:
                w1k = w1pool.tile([128, DT2, 128], dt.bfloat16, tag="w1k")
                nc.sync.dma_start(w1k[:], w1_d[k])
            psum1 = psA.tile([128, T + E], dt.float32, tag="psum1")
            for j in range(DT):
                nc.tensor.matmul(
                    psum1[:, 0:T], lhsT=w1k[:, j, :], rhs=xt[:, j * T : (j + 1) * T],
                    start=(j == 0), stop=(j == DT - 1),
                )
            for j in range(DT):
                nc.tensor.matmul(
                    psum1[:, T : T + E], lhsT=w1k[:, DT + j, :], rhs=xt[:, DT * T + j * E : DT * T + (j + 1) * E],
                    start=(j == 0), stop=(j == DT - 1),
                )
            tbk = tbpool.tile([128, NT], dt.bfloat16, tag="tb")
            tb32k = tbpool.tile([128, TA], dt.float32, tag="tb32")
            negak = npool.tile([128, E + NT], dt.bfloat16, tag="nega")
            if NEGA_FIRST:
                nc.scalar.activation(
                    negak[:, 0:E], psum1[:, T : T + E], act_fn.Copy, scale=-1.0
                )
                nc.scalar.activation(negak[:, E : E + NT], negak[:, 0:NT], act_fn.Copy)
                nc.scalar.activation(
                    tbk[:], psum1[:, 0:NT], act_fn.Identity, bias=b1sb[:, k : k + 1]
                )
                nc.scalar.activation(
                    tb32k[:], psum1[:, NT:T], act_fn.Identity, bias=b1sb[:, k : k + 1]
                )
            else:
                nc.scalar.activation(
                    tbk[:], psum1[:, 0:NT], act_fn.Identity, bias=b1sb[:, k : k + 1]
                )
                nc.scalar.activation(
                    tb32k[:], psum1[:, NT:T], act_fn.Identity, bias=b1sb[:, k : k + 1]
                )
                nc.scalar.activation(
                    negak[:, 0:E], psum1[:, T : T + E], act_fn.Copy, scale=-1.0
                )
                nc.scalar.activation(negak[:, E : E + NT], negak[:, 0:NT], act_fn.Copy)

            m = mpool.tile([128, MC], dt.bfloat16, tag="m")
            ta = tbk[:]
            na = negak[:]

            def emit_tt(r0, runs):
                cb = AP(
                    tensor=ta.tensor, offset=ta.offset,
                    ap=[[NT, 128], [0, runs], [1, NT]],
                )
                nd = AP(
                    tensor=na.tensor, offset=na.offset + r0,
                    ap=[[E + NT, 128], [1, runs], [1, NT]],
                )
                nc.vector.tensor_tensor(m[:, r0 * NT : (r0 + runs) * NT], cb, nd, alu.max)

            def emit_mm2(c0, c1):
                for c in range(c0, c1):
                    g = c % 4
                    nc.tensor.matmul(
                        ps2[c // 4][32 * g : 32 * g + 2, :],
                        lhsT=w2sb[:, k, :], rhs=m[:, c * CH : (c + 1) * CH],
                        start=(k == 0), stop=(k == HT - 1),
                        tile_position=(0, 32 * g),
                    )

            ha = apool.tile([128, TA * E], dt.bfloat16, tag="ha")
            for j in range(TA):
                nc.scalar.activation(
                    ha[:, j * E : (j + 1) * E], negak[:, 0:E], act_fn.Relu,
                    bias=tb32k[:, j : j + 1], scale=-1.0,
                )

            if SPLIT_LAST_TT and k == HT - 1:
                for part in range(3):
                    emit_tt(part * 32, 32)
                    emit_mm2(part * 3, part * 3 + 3)
            else:
                emit_tt(0, E)
                emit_mm2(0, NCH)
            nc.tensor.matmul(
                ps2[2][32:34, 0 : TA * E], lhsT=w2sb[:, k, :], rhs=ha[:],
                start=(k == 0), stop=(k == HT - 1),
                tile_position=(0, 32),
            )
            nc.tensor.matmul(
                ps2[2][64:66, 0:E], lhsT=w2sb[:, k, :], rhs=negak[:, 0:E],
                start=(k == 0), stop=(k == HT - 1),
                tile_position=(0, 64),
            )

        outm_sb = consts.tile([2, MC], dt.float32)
        outa_sb = consts.tile([2, TA * E], dt.float32)
        outv_sb = consts.tile([2, E], dt.float32)
        for c in range(NCH):
            g = c % 4
            src = ps2[c // 4][32 * g : 32 * g + 2, :]
            dst = outm_sb[:, c * CH : (c + 1) * CH]
            if c % 2 == 0:
                nc.vector.tensor_copy(dst, src)
            else:
                nc.scalar.activation(dst, src, act_fn.Copy)
            eng = nc.sync if c % 2 == 0 else nc.gpsimd
            eng.dma_start(outm_d[:, c * CH : (c + 1) * CH], dst)
        nc.scalar.activation(outa_sb[:], ps2[2][32:34, 0 : TA * E], act_fn.Copy)
        nc.vector.tensor_copy(outv_sb[:], ps2[2][64:66, 0:E])
        nc.gpsimd.dma_start(outa_d[:], outa_sb[:])
        nc.sync.dma_start(outv_d[:], outv_sb[:])

    _split_excess_waits(nc, mybir)
    return nc


def _prep_inputs(trig_embed, arg_embed, W1, b1, W2, b2):
    bf16 = ml_dtypes.bfloat16
    w1t = np.ascontiguousarray(
        W1.reshape(DT2, 128, HT, 128).transpose(2, 1, 0, 3)
    ).astype(bf16)
    w2t = np.ascontiguousarray(W2.reshape(HT, 128, O).transpose(1, 0, 2)).astype(bf16)
    b1t = np.ascontiguousarray(b1.reshape(HT, 128).T).astype(np.float32)
    in_maps = []
    for b in range(B):
        xtt = trig_embed[b].T.reshape(DT, 128, T).transpose(1, 0, 2).reshape(128, DT * T)
        xta = arg_embed[b].T.reshape(DT, 128, E).transpose(1, 0, 2).reshape(128, DT * E)
        xt = np.ascontiguousarray(np.concatenate([xtt, xta], axis=1)).astype(bf16)
        in_maps.append({"xt": xt, "w1t": w1t, "w2t": w2t, "b1t": b1t})
    return in_maps


def _colmap():
    r = np.arange(E)[:, None]
    i = np.arange(NT)[None, :]
    t = np.broadcast_to(i, (E, NT)).ravel()
    e = ((r + i) % E).ravel()
    return t, e


def run(inputs, trace=False):
    from concourse.bass_utils import run_bass_kernel_spmd

    if "nc" not in _cache:
        _cache["nc"] = _build_nc()
        _cache["cmap"] = _colmap()
    nc = _cache["nc"]
    t_idx, e_idx = _cache["cmap"]
    b2 = np.asarray(inputs["b2"], np.float32)
    in_maps = _prep_inputs(**inputs)
    res = run_bass_kernel_spmd(nc, in_maps, core_ids=list(range(B)), trace=trace)
    full = np.empty((B, T, E, O), np.float32)
    for b in range(B):
        outm = res.results[b]["outm"]
        outa = res.results[b]["outa"]
        outv = res.results[b]["outv"]
        fb = full[b]
        mm = outm - outv[:, e_idx]
        fb[t_idx, e_idx, 0] = mm[0]
        fb[t_idx, e_idx, 1] = mm[1]
        aa = outa.reshape(O, TA, E)
        fb[NT:T, :, 0] = aa[0]
        fb[NT:T, :, 1] = aa[1]
        fb += b2
    return full, res


def kernel(**inputs):
    full, _ = run(inputs, trace=False)
    return full



# revision 2
# speedup vs baseline: 1.0058x; 1.0058x over previous
"""Trainium2 Bass kernel v5: diagonal max pipeline, consolidated tiles.

Per core (one batch):
  mm1: psum1[h128, T+E] = W1.T @ x  (12 MMs per h-tile)
  X tile [128, 480] per h-tile: [ha 0:288 | nega 288:384 | wrap 384:435 | tb 435:480]
    nega = -(arg_h+b1) [ACT, bias folded]; wrap = nega[0:51] copy; tb = trig_h[:45] bf16
    ha_j = relu(-nega + tb32[j]) = hidden for assist t's  [ACT]
  DVE TT: m[h, col] = max(tb[t], nega[e]), col=(r,i): t=i, e=(r+i)%E  (diag)
  mm2: 9 chunk MMs over m + 1 MM over X[0:480] (ha|v|junk), col-tiled into one
    3-bank psum tile; slots (g,b) = [32g:32g+2, 512b:512b+480].
  Drain: 2 partition-strided copies [4, 1440] (rows o=0 / o=1) + 2 DMAs.
  host: out = outm - v + b2 (diag region), ha + b2 (assist region).
"""

import sys

if "/opt/trn_rl_repo" not in sys.path:
    sys.path.insert(0, "/opt/trn_rl_repo")

import numpy as np
import ml_dtypes

B, T, E, D, H, O = 8, 48, 96, 768, 2048, 2
HT = H // 128
DT2 = (2 * D) // 128
DT = D // 128
NT = 45                   # t's on the DVE diagonal path
TA = T - NT               # t's on the ACT-assist path
MC = E * NT              # diag columns per h-tile
CH = 480                  # mm2 chunk (MC = 9*480)
NCH = MC // CH
XW = 480                  # X tile: ha 288 | nega 96 | wrap 51 | tb 45
XN, XWRAP, XTB = TA * E, TA * E + E, TA * E + E + 51
BIGCOPY = False

_cache = {}


def _split_excess_waits(nc, mybir, max_waits=1):
    n_split = 0
    for f in nc.m.functions:
        for bb in f.blocks:
            new_insts = []
            for ins in bb.instructions:
                si = getattr(ins, "sync_info", None)
                ow = list(si.on_wait) if (si and si.on_wait) else []
                if len(ow) > max_waits:
                    head, rest = ow[:-max_waits], ow[-max_waits:]
                    for k in range(0, len(head), max_waits):
                        nop = mybir.InstNoOp(
                            name=nc.get_next_instruction_name(), ins=[], outs=[]
                        )
                        nop.engine = ins.engine
                        nop.sync_info = mybir.SyncInfo(
                            on_wait=head[k : k + max_waits], on_update=[]
                        )
                        nop.bass_nofuse = True
                        new_insts.append(nop)
                        n_split += 1
                    si.on_wait = rest
                new_insts.append(ins)
            bb.instructions[:] = new_insts
    return n_split


def _build_nc():
    import concourse.bass as bass
    import concourse.mybir as mybir
    import concourse.tile as tile
    from concourse.bass import AP
    from contextlib import ExitStack

    dt = mybir.dt
    alu = mybir.AluOpType
    act_fn = mybir.ActivationFunctionType

    nc = bass.Bass()
    xt_d = nc.declare_dram_parameter("xt", [128, DT * (T + E)], dt.bfloat16, isOutput=False)
    w1_d = nc.declare_dram_parameter("w1t", [HT, 128, DT2, 128], dt.bfloat16, isOutput=False)
    w2_d = nc.declare_dram_parameter("w2t", [128, HT, O], dt.bfloat16, isOutput=False)
    b1_d = nc.declare_dram_parameter("nb1t", [128, HT], dt.float32, isOutput=False)
    stg0_d = nc.declare_dram_parameter("stg0", [O, 5 * CH], dt.float32, isOutput=True)
    stg1_d = nc.declare_dram_parameter("stg1", [O, 5 * CH], dt.float32, isOutput=True)

    with ExitStack() as ctx:
        tc = ctx.enter_context(tile.TileContext(nc))
        consts = ctx.enter_context(tc.tile_pool(name="consts", bufs=1))
        w1pool = ctx.enter_context(tc.tile_pool(name="w1pool", bufs=4))
        tbpool = ctx.enter_context(tc.tile_pool(name="tbpool", bufs=6))
        xpool = ctx.enter_context(tc.tile_pool(name="xpool", bufs=6))
        mpool = ctx.enter_context(tc.tile_pool(name="mpool", bufs=4))
        psA = ctx.enter_context(tc.tile_pool(name="psA", bufs=3, space="PSUM"))
        psB = ctx.enter_context(tc.tile_pool(name="psB", bufs=1, space="PSUM"))

        xt = consts.tile([128, DT * (T + E)], dt.bfloat16)
        w2sb = consts.tile([128, HT, O], dt.bfloat16)
        b1sb = consts.tile([128, HT], dt.float32)

        # Dependency-free dummy Relu: hoists the 1.3us ACT_TABLE_LOAD into
        # the preamble/DMA dead time instead of blocking the first drain.
        warm = consts.tile([128, 2], dt.bfloat16)
        nc.vector.memset(warm[:, 0:1], 0.0)
        nc.scalar.activation(warm[:, 1:2], warm[:, 0:1], act_fn.Relu)

        # Head: 3 big prefetch DMAs, one per queue.
        nc.sync.dma_start(xt[:], xt_d[:])
        w1k0 = w1pool.tile([128, DT2, 128], dt.bfloat16, tag="w1k")
        nc.scalar.dma_start(w1k0[:], w1_d[0])
        nc.gpsimd.dma_start(b1sb[:], b1_d[:])
        nc.gpsimd.dma_start(w2sb[:], w2_d[:])

        # Three 1-bank psum tiles; slot i: tile i//4, rows 32(i%4):+2.
        ps2 = [psB.tile([128, CH], dt.float32, name=f"ps2_{i}", tag=f"ps2_{i}") for i in range(3)]

        def slot(i):
            g = i % 4
            return ps2[i // 4][32 * g : 32 * g + 2, :], (0, 32 * g)

        SLOT_AV = 9

        for k in range(HT):
            if k == 0:
                w1k = w1k0
            else:
                w1k = w1pool.tile([128, DT2, 128], dt.bfloat16, tag="w1k")
                q = nc.sync if k % 2 else nc.scalar
                q.dma_start(w1k[:], w1_d[k])
            psum1 = psA.tile([128, T + E], dt.float32, tag="psum1")
            for j in range(DT):
                nc.tensor.matmul(
                    psum1[:, 0:T], lhsT=w1k[:, j, :], rhs=xt[:, j * T : (j + 1) * T],
                    start=(j == 0), stop=(j == DT - 1),
                )
            for j in range(DT):
                nc.tensor.matmul(
                    psum1[:, T : T + E], lhsT=w1k[:, DT + j, :], rhs=xt[:, DT * T + j * E : DT * T + (j + 1) * E],
                    start=(j == 0), stop=(j == DT - 1),
                )
            X = xpool.tile([128, XW], dt.bfloat16, tag="X")
            tb32k = tbpool.tile([128, TA], dt.float32, tag="tb32")
            # nega = -(a + b1): scale=-1, bias=-b1 (host-negated)
            nc.scalar.activation(
                X[:, XN : XN + E], psum1[:, T : T + E], act_fn.Identity,
                scale=-1.0, bias=b1sb[:, k : k + 1],
            )
            # wrap-extension (51 cols) for the diagonal reads
            nc.scalar.activation(X[:, XWRAP:XTB], X[:, XN : XN + 51], act_fn.Copy)
            # c (trig_h) copies: bf16 for the diag TT, f32 for ha biases
            nc.scalar.activation(X[:, XTB:XW], psum1[:, 0:NT], act_fn.Copy)
            nc.scalar.activation(tb32k[:], psum1[:, NT:T], act_fn.Copy)

            m = mpool.tile([128, MC], dt.bfloat16, tag="m")

            def emit_tt(r0, runs):
                cb = AP(
                    tensor=X.tensor, offset=X.offset + XTB,
                    ap=[[XW, 128], [0, runs], [1, NT]],
                )
                nd = AP(
                    tensor=X.tensor, offset=X.offset + XN + r0,
                    ap=[[XW, 128], [1, runs], [1, NT]],
                )
                nc.vector.tensor_tensor(m[:, r0 * NT : (r0 + runs) * NT], cb, nd, alu.max)

            def emit_mm2(c0, c1):
                for c in range(c0, c1):
                    dst, tp = slot(c)
                    nc.tensor.matmul(
                        dst, lhsT=w2sb[:, k, :], rhs=m[:, c * CH : (c + 1) * CH],
                        start=(k == 0), stop=(k == HT - 1), tile_position=tp,
                    )

            for j in range(TA):
                nc.scalar.activation(
                    X[:, j * E : (j + 1) * E], X[:, XN : XN + E], act_fn.Relu,
                    bias=tb32k[:, j : j + 1], scale=-1.0,
                )

            if k == HT - 1:
                # pieces aligned so chunk MMs/drains start before the TT ends
                bounds = [0, 22, 43, 64, 86, E]
                cov = [2, 4, 6, 8, 9]
                c0 = 0
                for p in range(5):
                    emit_tt(bounds[p], bounds[p + 1] - bounds[p])
                    emit_mm2(c0, cov[p])
                    c0 = cov[p]
            else:
                emit_tt(0, E)
                emit_mm2(0, NCH)
            dst, tp = slot(SLOT_AV)
            nc.tensor.matmul(
                dst, lhsT=w2sb[:, k, :], rhs=X[:],
                start=(k == 0), stop=(k == HT - 1), tile_position=tp,
            )

        # Drain: one [2, CH] copy per slot into a single staging tile.
        # Scalar drains early chunks (ready while the split TT_15 still
        # runs); Vector drains the late ones after its last TT piece.
        stg = consts.tile([2, NCH * CH + CH], dt.float32)

        def drain(i, on_scalar):
            reg, _ = slot(i)
            w = CH if i != SLOT_AV else TA * E + E
            dst = stg[:, i * CH : i * CH + w]
            if on_scalar:
                nc.scalar.activation(dst, reg[:, 0:w], act_fn.Copy)
            else:
                nc.vector.tensor_copy(dst, reg[:, 0:w])

        for i in (0, 1, 2, 3, SLOT_AV, 4, 5):
            drain(i, True)
        for i in (6, 7, 8):
            drain(i, False)
        nc.sync.dma_start(stg0_d[:], stg[:, 0 : 5 * CH])
        nc.scalar.dma_start(stg1_d[:], stg[:, 5 * CH :])

    _split_excess_waits(nc, mybir)
    return nc


def _prep_inputs(trig_embed, arg_embed, W1, b1, W2, b2):
    bf16 = ml_dtypes.bfloat16
    w1t = np.ascontiguousarray(
        W1.reshape(DT2, 128, HT, 128).transpose(2, 1, 0, 3)
    ).astype(bf16)
    w2t = np.ascontiguousarray(W2.reshape(HT, 128, O).transpose(1, 0, 2)).astype(bf16)
    nb1t = np.ascontiguousarray(-b1.reshape(HT, 128).T).astype(np.float32)
    in_maps = []
    for b in range(B):
        xtt = trig_embed[b].T.reshape(DT, 128, T).transpose(1, 0, 2).reshape(128, DT * T)
        xta = arg_embed[b].T.reshape(DT, 128, E).transpose(1, 0, 2).reshape(128, DT * E)
        xt = np.ascontiguousarray(np.concatenate([xtt, xta], axis=1)).astype(bf16)
        in_maps.append({"xt": xt, "w1t": w1t, "w2t": w2t, "nb1t": nb1t})
    return in_maps


def _colmap():
    r = np.arange(E)[:, None]
    i = np.arange(NT)[None, :]
    t = np.broadcast_to(i, (E, NT)).ravel()
    e = ((r + i) % E).ravel()
    return t, e


def run(inputs, trace=False):
    from concourse.bass_utils import run_bass_kernel_spmd

    if "nc" not in _cache:
        _cache["nc"] = _build_nc()
        _cache["cmap"] = _colmap()
    nc = _cache["nc"]
    t_idx, e_idx = _cache["cmap"]
    b2 = np.asarray(inputs["b2"], np.float32)
    in_maps = _prep_inputs(**inputs)
    res = run_bass_kernel_spmd(nc, in_maps, core_ids=list(range(B)), trace=trace)
    full = np.empty((B, T, E, O), np.float32)
    for b in range(B):
        stg = np.concatenate(
            [res.results[b]["stg0"], res.results[b]["stg1"]], axis=1
        )  # [O, 10*CH]: chunks 0-8 diag, slot 9 av
        outm = stg[:, 0:MC]
        outa = stg[:, NCH * CH : NCH * CH + TA * E]
        outv = stg[:, NCH * CH + TA * E : NCH * CH + TA * E + E]
        fb = full[b]
        mm = outm - outv[:, e_idx]
        fb[t_idx, e_idx, 0] = mm[0]
        fb[t_idx, e_idx, 1] = mm[1]
        aa = outa.reshape(O, TA, E)
        fb[NT:T, :, 0] = aa[0]
        fb[NT:T, :, 1] = aa[1]
        fb += b2
    return full, res


def kernel(**inputs):
    full, _ = run(inputs, trace=False)
    return full
